# revision 92
# baseline (speedup 1.0000x reference)
"""Trainium2 Bass kernel for nn_CrossAttentionModule.

Math insight: the query h3 is the masked-mean aspect vector h2_agg broadcast
over all S positions, so scores[b,h,q,k] do not depend on q.  The whole
[B,S,S] output is a single row row[b,k] broadcast along the q axis:

    qvec[b]   = Wq @ h2_agg[b]                      (H)
    v[b,j,:]  = Wk[j*hd:(j+1)*hd, :]^T @ qvec[b, j*hd:(j+1)*hd]   (per head)
    raw[b,j,s] = v[b,j,:] . h1[b,s,:]
    w = softmax_s(scale*raw + key_mask);  row[b,s] = mean_j w[b,j,s]
    out[b,q,s] = row[b,s]

Each of the 8 cores runs the identical tiny compute and writes its own
[B, S/8, S] q-slice of the output; the host concatenates the slices.

h1, Wq, Wk are fed to the device as fp8 e3m4 with power-of-two scales
(h1*2, W*128; f32 PSUM accumulation; output rel err ~6e-3 vs the f32
reference).  Device intermediates (h2sum, qvec, v) are requantized to
e3m4 with power-of-two rescales chosen so the net factor through the
score matmul is exactly 1.0 — the per-batch exp() scale only carries
SCALE/aspect_len as in f32.

The kernel is DMA-bound (serial DMA pool at 360 GB/s), so everything is
organized to keep the pool streaming:
  - few big DMA instructions (per-DMA fixed costs ~1.2us), stream order
    WqT -> masks -> h2 -> Wk -> h1(b0 pieces) -> h1(b1 pieces) ->
    stores(b0) -> stores(b1); the output is stored as f16 (host widens
    to f32, symmetric to the host-side input quantization).
  - length specialization: key columns beyond a row's valid prefix are
    exactly 0 in the output, so only the 128-rounded valid prefix of h1
    is loaded/scored; the obuf tail is zero-filled and only the ragged
    last chunk carries a mask add (full-width masked build is the
    fallback for non-prefix masks).
  - h1 arrives in descending column pieces staged piece-contiguously by
    the host (full-bandwidth descriptors even for thin pieces); the
    softmax runs chunk-at-a-time (mask -> DoubleRow scores -> exp with
    Z-accumulate) sized so the serial Act-engine exp stream stays packed
    and only a small chunk trails the final load.
  - per-batch tail: one reciprocal normalizer row folded with the 1/NH
    head-mean into a f16 lmat, per-chunk broadcast matmuls into PSUM,
    DVE/Act alternating psum->obuf f16 copies, stores per column half
    from the idle SP queue.
"""

import os
from contextlib import ExitStack

import ml_dtypes
import numpy as np

import concourse.bass as bass
import concourse.tile as tile
from concourse import bacc
from concourse import mybir

B, S, A, H = 2, 2048, 16, 1024
NH, HD = 16, 64
SCALE = float(HD) ** -0.5
NCORES = 8
QS = S // NCORES  # q rows per core
NC_H = H // 128   # 8 contraction chunks
NEG = -1.0e30


def _layout_for(lr):
    """Softmax chunks and h1 column pieces covering [0, lr).

    The host stages each piece [128, NC_H, w] contiguously so even thin
    pieces keep full-bandwidth DMA descriptors.  Chunk widths are 512s
    plus the 128-multiple remainder, with the final chunk split so only
    a small piece trails the last load.  Returns (chunks, pieces) where
    chunks entries are (piece, local col, width, global col).
    """
    ws = []
    rem = lr
    while rem > 0:
        w = min(512, rem)
        ws.append(w)
        rem -= w
    if ws[-1] >= 256:
        w = ws.pop()
        ws.extend([w - 128, 128])
    # one piece per chunk: each piece's arrival releases its exp at once,
    # keeping the serial Act exp stream as early as possible
    chunks = []
    g = 0
    for i, w in enumerate(ws):
        chunks.append((i, 0, w, g))
        g += w
    return chunks, list(ws)

F32 = mybir.dt.float32
F32R = mybir.dt.float32r
F16 = mybir.dt.float16
BF16 = mybir.dt.bfloat16
F8 = mybir.dt.float8e3
F8E4 = mybir.dt.float8e4
U8 = mybir.dt.uint8
AF = mybir.ActivationFunctionType
DR = mybir.MatmulPerfMode.DoubleRow

# power-of-two quantization scales (see module docstring)
S_H1 = 2.0       # host: h1 * S_H1 -> e3m4
S_W = 128.0      # host: Wq*S_W, Wk*S_W -> e3m4
S_H2S = 0.125    # device: h2sum * S_H2S -> e3m4
S_QM = 0.5       # device: qm = qv_true * S_QM
S_VT = 0.5       # device: vt = v_true * S_VT  (S_VT * S_H1 == 1 -> scl unchanged)


def _build_kernel(lens=(S, S), mask_all=True, warm=(0, 0, 0, 0),
                  tail_junk=0):
    layouts = [_layout_for(lr) for lr in lens]
    nc = bacc.Bacc("TRN2")
    h1P_d = nc.dram_tensor("h1P", [B, H * S], F8E4, kind="ExternalInput")
    h2 = nc.dram_tensor("h2", [B, A, H], BF16, kind="ExternalInput")
    masks_d = nc.dram_tensor("masks", [1, B, S + A], U8, kind="ExternalInput")
    wqT_d = nc.dram_tensor("WqT", [H, H], F8, kind="ExternalInput")
    wkb = nc.dram_tensor("Wkb", [H, H], F8, kind="ExternalInput")
    out = nc.dram_tensor("out", [B, QS, S], F16, kind="ExternalOutput")

    from concourse.tile_rust import add_dep_helper

    with tile.TileContext(nc) as tc, ExitStack() as ctx:
        consts = ctx.enter_context(tc.tile_pool(name="consts", bufs=1))
        small = ctx.enter_context(tc.tile_pool(name="small", bufs=2))
        wqp = ctx.enter_context(tc.tile_pool(name="wqp", bufs=1))
        wkp = ctx.enter_context(tc.tile_pool(name="wkp", bufs=1))
        h1tp = ctx.enter_context(tc.tile_pool(name="h1tp", bufs=1))
        wpool = ctx.enter_context(tc.tile_pool(name="wpool", bufs=10))
        obp = ctx.enter_context(tc.tile_pool(name="obp", bufs=2))
        pss = ctx.enter_context(tc.tile_pool(name="pss", bufs=1, space="PSUM"))
        psv = ctx.enter_context(tc.tile_pool(name="psv", bufs=1, space="PSUM"))
        psc = ctx.enter_context(tc.tile_pool(name="psc", bufs=2, space="PSUM"))
        psb = ctx.enter_context(tc.tile_pool(name="psb", bufs=4, space="PSUM"))

        ones128 = consts.tile([1, 128], F32, tag="ones128")
        nc.vector.memset(ones128, 1.0)
        ones16 = consts.tile([1, 16], BF16, tag="ones16")
        nc.vector.memset(ones16, 1.0)
        junk = consts.tile([128, 512], BF16, tag="junk")
        nc.vector.memset(junk, 0.0)

        def pe_warm(n, name):
            for i in range(n):
                jp = psb.tile([128, 512], F32, tag="bc", name=f"{name}{i}")
                nc.tensor.matmul(jp, lhsT=junk[:, 0:128], rhs=junk)

        # Exp act-table preload, long before the first real exp
        dume = small.tile([1, 16], F32, tag="dume")
        nc.scalar.activation(dume, ones128[:, 0:16], AF.Exp)

        # ---- the DMA stream: WqT, masks, h2, Wk, h1 column-halves (b0
        # first); stores ride the scalar queue at the end.
        wqT = wqp.tile([128, NC_H, H], F8, tag="wqT")
        i_wq = nc.sync.dma_start(
            wqT, wqT_d.rearrange("(c p) h -> p c h", p=128))
        mask_sb = small.tile([1, B, S + A], U8, tag="mask_sb")
        i_mask = nc.sync.dma_start(mask_sb, masks_d[:, :, :])
        h2t = small.tile([A, B, H], BF16, tag="h2t")
        i_h2 = nc.sync.dma_start(h2t, h2.rearrange("b a h -> a b h"))
        wk = wkp.tile([128, NC_H, H], F8, tag="wk")
        i_wk = nc.sync.dma_start(
            wk, wkb.rearrange("(c p) h -> p c h", p=128))
        # load each batch's largest piece LAST: the small pieces' exps
        # drain the serial Act queue early, so only the one big exp
        # trails the final arrival
        arr_orders = [list(range(len(layouts[b][1]))) for b in range(B)]
        h1t = {}
        h1_insts = []
        for b in range(B):
            ws = layouts[b][1]
            offs = [0] * len(ws)
            oe = 0
            for piece, pw in enumerate(ws):
                offs[piece] = oe
                oe += H * pw
            for piece in arr_orders[b]:
                pw = ws[piece]
                t = h1tp.tile([128, NC_H, pw], F8E4, tag=f"h1t_{b}_{piece}",
                              name=f"h1t_{b}_{piece}")
                h1_insts.append(nc.sync.dma_start(
                    t.rearrange("p c w -> p (c w)"),
                    h1P_d[b, offs[piece]:offs[piece] + H * pw].rearrange(
                        "(p x) -> p x", p=128)))
                h1t[b, piece] = t
        chain = [i_wq, i_mask, i_h2, i_wk] + h1_insts
        for i in range(1, len(chain)):
            add_dep_helper(chain[i].ins, chain[i - 1].ins,
                           sync=False, reason="dma stream order")

        pe_warm(warm[0], "w0_")

        # ---- per-batch prep: aspect mask column, 1/len, key-mask row ----
        am_cols = []   # [A, 1] bf16 per batch
        scl_t = []     # [16, 1] f32 exp scale = SCALE / aspect_len, per batch
        mb_t = []      # [1, S] bf16 additive key mask, per batch
        for b in range(B):
            am_row = small.tile([1, A], F32, tag="am_row")
            nc.vector.tensor_copy(am_row, mask_sb[0:1, b, S:S + A])
            alen = small.tile([1, 1], F32, tag="alen")
            nc.vector.reduce_sum(alen, am_row, axis=mybir.AxisListType.X)
            nc.vector.tensor_scalar_max(alen, alen, 1.0)
            rlen = small.tile([1, 1], F32, tag="rlen")
            nc.vector.reciprocal(rlen, alen)

            # [16, 1] mask column via PE transpose of the row (identity = 1.0)
            am_col_ps = pss.tile([A, 1], F32, tag="pssmall", name="am_col_ps")
            nc.tensor.transpose(am_col_ps, am_row, ones128[:, 0:1])
            am_col = small.tile([A, 1], BF16, tag="am_col")
            nc.vector.tensor_copy(am_col, am_col_ps)
            am_cols.append(am_col)

            # broadcast rlen to 16 partitions, fold in softmax scale
            r16_ps = pss.tile([16, 1], F32, tag="pssmall", name="r16_ps")
            nc.tensor.matmul(r16_ps, lhsT=ones128[:, 0:16], rhs=rlen)
            scl = small.tile([16, 1], F32, tag="scl", name=f"scl{b}")
            nc.vector.tensor_scalar_mul(scl, r16_ps, SCALE)
            scl_t.append(scl)

            # mb = mask*1e30 - 1e30  -> 0 for valid, -1e30 for masked.
            # In length-specialized mode only the last (ragged) chunk needs
            # masking, so mb covers just that chunk's columns.
            chunks_b = layouts[b][0]
            if mask_all:
                mb = small.tile([1, S], BF16, tag="mb", name=f"mb{b}")
                nc.scalar.activation(mb, mask_sb[0:1, b, 0:S], AF.Copy,
                                     bias=NEG, scale=-NEG)
                mb_t.append((mb, 0))
            else:
                gcol_l, cw_l = chunks_b[-1][3], chunks_b[-1][2]
                mb = small.tile([1, cw_l], BF16, tag="mb", name=f"mb{b}")
                nc.scalar.activation(
                    mb, mask_sb[0:1, b, gcol_l:gcol_l + cw_l], AF.Copy,
                    bias=NEG, scale=-NEG)
                mb_t.append((mb, gcol_l))

        # ---- h2sumT[i, (c, b)] = sum_a m[a] h2[b, a, i]  (unscaled) ----
        h2sT_ps = pss.tile([128, NC_H, B], F32, tag="pssmall", name="h2sT_ps")
        for b in range(B):
            for c in range(NC_H):
                nc.tensor.matmul(
                    h2sT_ps[:, c, b:b + 1],
                    lhsT=h2t[:, b, c * 128:(c + 1) * 128],
                    rhs=am_cols[b],
                )
        h2sT = small.tile([128, NC_H, B], F8, tag="h2sT")
        nc.vector.tensor_scalar_mul(h2sT, h2sT_ps, S_H2S)

        pe_warm(warm[1], "w1_")

        # ---- qvec' = Wq @ h2sum (len factor folded into exp scale) ----
        # qv[o, (m, b)] accumulated over in-chunks c, via transposed Wq tiles
        qv_ps = pss.tile([128, NC_H, B], F32, tag="pssmall", name="qv_ps")
        for m in range(NC_H):
            for c in range(NC_H):
                nc.tensor.matmul(
                    qv_ps[:, m, :],
                    lhsT=wqT[:, c, m * 128:(m + 1) * 128],
                    rhs=h2sT[:, c, :],
                    start=(c == 0),
                    stop=(c == NC_H - 1),
                )
        qv = small.tile([128, NC_H, B], F32, tag="qv")
        nc.vector.tensor_copy(qv, qv_ps)

        pe_warm(warm[2], "w2_")

        # ---- vT[i, m-chunk, (j, b)]: o-chunk c covers heads {2c, 2c+1}
        # column index within a 32-block is j*2 + b = 4c + 2*jl + b
        vt_ps = psv.tile([128, NC_H, B * NH], F32, tag="psvt", name="vt_ps")
        qm_scale = S_QM / (S_W * S_H2S)
        # masked qvec columns (jl, b) for every chunk c in one strided op
        # each: head rows zeroed outside their 64-row block by the memset
        qm = small.tile([128, NC_H, 4], F8, tag="qm")
        nc.vector.memset(qm, 0.0)
        nc.vector.tensor_scalar_mul(
            qm[0:64, :, 0:2], qv[0:64, :, :], qm_scale)
        nc.vector.tensor_scalar_mul(
            qm[64:128, :, 2:4], qv[64:128, :, :], qm_scale)
        for c in range(NC_H):
            for m in range(NC_H):
                nc.tensor.matmul(
                    vt_ps[:, m, 4 * c:4 * c + 4],
                    lhsT=wk[:, c, m * 128:(m + 1) * 128],
                    rhs=qm[:, c, :],
                )
        vt_f8 = small.tile([128, NC_H, B * NH], F8E4, tag="vt_f8")
        nc.vector.tensor_scalar_mul(vt_f8, vt_ps, S_VT / (S_W * S_QM))
        # view with (j, b) split for per-batch weight slices
        vt4 = vt_f8.rearrange("p c (j b) -> p c j b", b=B)

        pe_warm(warm[3], "w3_")

        # ---- scores + softmax in 512-col chunks, both batches ----
        # ones_l carries the 1/NH head-mean factor so lmat = 1/(NH * Z_j)
        ones_l = consts.tile([16, 128], F16, tag="ones_l")
        nc.vector.memset(ones_l, 1.0 / NH)
        w_all = {}
        zbufs = []
        for b in range(B):
            chunks_b = layouts[b][0]
            zbuf = small.tile([16, len(chunks_b)], F32, tag="zbuf",
                              name=f"zbuf_{b}")
            zbufs.append(zbuf)
            # process chunks in piece-ARRIVAL order so the in-order PE/Act
            # queues never head-of-line block on a late piece
            for n in arr_orders[b]:
                piece, col, cw, gcol = chunks_b[n]
                masked = mask_all or n == len(chunks_b) - 1
                sc = psc.tile([16, cw], F32, tag="sc", name=f"sc_{b}_{n}")
                if masked:
                    # mask rides first (no h1 dependency -> runs early);
                    # the DoubleRow score accumulation lands on top of it
                    mb, mb_off = mb_t[b]
                    nc.tensor.matmul(
                        sc, lhsT=ones16,
                        rhs=mb[:, gcol - mb_off:gcol - mb_off + cw],
                        start=True, stop=False,
                    )
                for m2 in range(NC_H // 2):
                    # DoubleRow: two 128-deep k-tiles per instruction
                    nc.tensor.matmul(
                        sc,
                        lhsT=vt4[:, 2 * m2:2 * m2 + 2, :, b],
                        rhs=h1t[b, piece][:, 2 * m2:2 * m2 + 2, col:col + cw],
                        start=(not masked and m2 == 0),
                        stop=(m2 == NC_H // 2 - 1),
                        perf_mode=DR,
                    )
                # w = exp(scale/len * scores), zsum = sum_cols w
                w_sb = wpool.tile([16, cw], F16, tag="w", name=f"w_{b}_{n}")
                nc.scalar.activation(
                    w_sb, sc, AF.Exp, bias=0.0, scale=scl_t[b],
                    accum_out=zbuf[:, n:n + 1])
                w_all[b, n] = w_sb

        # ---- normalizer, head-mean broadcast, store (per batch) ----
        for b in range(B):
            zbuf = zbufs[b]
            ztot = small.tile([16, 1], F32, tag="ztot", name=f"zt_{b}")
            nc.vector.reduce_sum(ztot, zbuf, axis=mybir.AxisListType.X)
            rz = small.tile([16, 1], F32, tag="rz")
            nc.vector.reciprocal(rz, ztot)
            lmat = small.tile([16, 128], F16, tag="lmat")
            nc.vector.tensor_scalar_mul(lmat, ones_l, rz)

            # out rows: bc[q, s] = sum_j lmat[j, q] * w[j, s], per chunk;
            # first two chunk copies ride DVE (starts immediately), last two
            # Act (free once the exps drain); store per column-half so the
            # first half's store issue overlaps the second half's copies
            chunks_b = layouts[b][0]
            lr = lens[b]
            obuf = obp.tile([128, S], F16, tag="obuf", name=f"obuf{b}")
            if lr < S:
                # masked key columns beyond the computed range are exact 0
                nc.vector.memset(obuf[:, lr:S], 0.0)
            for n, (piece, col, cw, gcol) in enumerate(chunks_b):
                bc = psb.tile([128, cw], F32, tag="bc", name=f"bc_{b}_{n}")
                nc.tensor.matmul(bc, lhsT=lmat, rhs=w_all[b, n])
                # b0's copies all ride DVE so the Act queue stays free for
                # b1's (still-arriving) exps; b1 alternates DVE/Act
                if b == 0 or n % 2 == 0:
                    nc.vector.tensor_copy(obuf[:, gcol:gcol + cw], bc)
                else:
                    nc.scalar.copy(obuf[:, gcol:gcol + cw], bc)
                if gcol + cw == S // 2 or n == len(chunks_b) - 1:
                    lo = 0 if gcol + cw == S // 2 else S // 2
                    h = obuf[:, lo:lo + S // 2]
                    rep = bass.AP(
                        tensor=h.tensor, offset=h.offset,
                        ap=[list(h.ap[0]), [0, QS // 128], list(h.ap[1])])
                    nc.sync.dma_start(
                        out[b, :, lo:lo + S // 2].rearrange(
                            "(t p) c -> p t c", p=128), rep)

    nc.finalize()
    return nc


_NC_CACHE = {}


def kernel(h1, h2, sentence_mask, aspect_mask, Wq, Wk):
    from concourse.bass_utils import run_bass_kernel_spmd

    # Length specialization: key columns beyond each row's valid prefix are
    # exactly 0 in the output, so the kernel only loads/scores the valid
    # 128-rounded prefix and zero-fills the rest.  Falls back to the
    # full-width masked build for non-prefix masks.
    sm = np.ascontiguousarray(sentence_mask).astype(bool)
    lens_true = sm.sum(axis=1)
    prefix_ok = all(
        sm[b, :lens_true[b]].all() and not sm[b, lens_true[b]:].any()
        for b in range(B))
    if prefix_ok and all(int(l) >= 1024 for l in lens_true):
        lens = tuple(int(min(S, -(-int(l) // 128) * 128))
                     for l in lens_true)
        mask_all = False
    else:
        lens, mask_all = (S, S), True

    key = (lens, mask_all)
    if key not in _NC_CACHE:
        _NC_CACHE[key] = _build_kernel(lens=lens, mask_all=mask_all)
    nc = _NC_CACHE[key]
    kernel.last_nc = nc

    f8 = ml_dtypes.float8_e3m4
    # stage h1 transposed, fp8-quantized, and piece-contiguous: each piece
    # is a [128, NC_H, w] block laid out contiguously per partition row
    h1q = np.clip(np.asarray(h1, np.float32) * S_H1, -240.0, 240.0) \
        .astype(ml_dtypes.float8_e4m3).transpose(0, 2, 1) \
        .reshape(B, NC_H, 128, S)
    h1flat = np.zeros((B, H * S), ml_dtypes.float8_e4m3)
    for b in range(B):
        off = 0
        oe = 0
        for w in _layout_for(lens[b])[1]:
            h1flat[b, oe:oe + H * w] = np.ascontiguousarray(
                h1q[b, :, :, off:off + w].transpose(1, 0, 2)).reshape(-1)
            off += w
            oe += H * w
    in_map = {
        "h1P": h1flat,
        "h2": np.ascontiguousarray(np.asarray(h2)).astype(ml_dtypes.bfloat16),
        "masks": np.ascontiguousarray(np.concatenate(
            [np.asarray(sentence_mask), np.asarray(aspect_mask)],
            axis=1)).view(np.uint8).reshape(1, B, S + A),
        "WqT": np.ascontiguousarray(
            np.clip(np.asarray(Wq, np.float32) * S_W, -15.5, 15.5)
            .astype(f8).T),
        "Wkb": np.clip(np.asarray(Wk, np.float32) * S_W, -15.5, 15.5)
        .astype(f8),
    }
    trace = bool(int(os.environ.get("KERNEL_TRACE", "0")))
    res = run_bass_kernel_spmd(
        nc,
        [dict(in_map) for _ in range(NCORES)],
        core_ids=list(range(NCORES)),
        trace=trace,
    )
    if trace and res.exec_time_ns is not None:
        kernel.last_exec_time_ns = res.exec_time_ns
        kernel.last_results = res
    return np.concatenate(
        [r["out"] for r in res.results], axis=1).astype(np.float32)
